# revision 26
# baseline (speedup 1.0000x reference)
"""Multi-head attention (B=4, S=2048, D=1024, H=16) on 8 Trainium2 NeuronCores.

Sharding: core c handles batch c//2 and head-group c%2 (8 heads = 512 dims of
the per-head concat). Each core computes its q/k/v projections (tensor
parallel over heads), attention for its 8 heads, and a partial output
projection over its 512 concat dims; the host sums the two partials per batch.

v3 dataflow (all matmuls bf16, f32 psum accumulate):
  - scores computed transposed S^T[k, q] so the softmax mask/bias is a
    per-partition ACT bias and exp(scale*s + bias) is one ACT op; the two
    K=64 head-halves run row-packed (concurrent row groups).
  - ctx^T = [V | 1]^T @ e accumulated over k-chunks, M=65: psum row 64 is
    the softmax denominator (ones column rides the contraction for free).
  - ctx matmuls lag the exp by TWO k-chunks so the PE stream never waits
    on the ACT semaphore (scores for kc are emitted ahead of ctx(kc-2)).
  - k/v/q projections and the transposed output projection run as
    background generators inside the attention loop; unit closes
    (normalization) are deferred into the next unit's first k-chunks so
    the ACT queue never drains at unit boundaries.
  - exp instructions are the serial resource: 256 x [128,1024] on the ACT
    engine (~1.1us each) bound the kernel; everything else hides under it.

PSUM banks: scores 2x[128,1024]=4, ctx pair (M=65) 2, proj 2.

Host epilogue: out[b] = partial[2b] + partial[2b+1] + (Wo @ bv + bo); the
value bias commutes with softmax (rows sum to 1) so it is exact. Key/query
biases applied on-device.
"""

import sys

sys.path.insert(0, "/opt/trn_rl_repo")

import numpy as np

import concourse.bacc as bacc
import concourse.mybir as mybir
import concourse.tile as tile
from concourse.bass_utils import run_bass_kernel_spmd

f32 = mybir.dt.float32
bf16 = mybir.dt.bfloat16
AF = mybir.ActivationFunctionType

B, S, E, H = 4, 2048, 1024, 16
DH = E // H  # 64
G = E // 2  # 512 dims per core (8 heads)
HL = H // 2  # heads per core
EC = E // 128  # 8 e-chunks (projection contraction)
DC = G // 128  # 4 head-pairs per core
QT = S // 512  # 4 q-tiles
KC = S // 128  # 16 k-chunks
GC = G // 128  # 4 chunks of the local concat dim (out-proj contraction)
SCALE = 1.0 / np.sqrt(np.float64(E))
MASK_NEG = -88.0  # exp(-88 + |s|max) == 0 in fp32 for masked keys

_NC = None


def _build_program():
    nc = bacc.Bacc("TRN2", target_bir_lowering=False, debug=False, num_devices=8)

    xqT = nc.dram_tensor("xqT", [E, S], bf16, kind="ExternalInput").ap()
    xkT = nc.dram_tensor("xkT", [E, S], bf16, kind="ExternalInput").ap()
    xvT = nc.dram_tensor("xvT", [E, S], bf16, kind="ExternalInput").ap()
    wqT = nc.dram_tensor("wqT", [E, G], bf16, kind="ExternalInput").ap()
    wkT = nc.dram_tensor("wkT", [E, G], bf16, kind="ExternalInput").ap()
    wvT = nc.dram_tensor("wvT", [E, G], bf16, kind="ExternalInput").ap()
    woT = nc.dram_tensor("woT", [G, E], bf16, kind="ExternalInput").ap()
    bqd = nc.dram_tensor("bqd", [128, DC], f32, kind="ExternalInput").ap()
    bkd = nc.dram_tensor("bkd", [128, DC], f32, kind="ExternalInput").ap()
    maskb = nc.dram_tensor("maskb", [128, KC], f32, kind="ExternalInput").ap()
    out = nc.dram_tensor("out", [E, S], f32, kind="ExternalOutput").ap()  # transposed

    with tile.TileContext(nc) as tc:
        with (
            tc.tile_pool(name="weights", bufs=1) as wpool,
            tc.tile_pool(name="persist", bufs=1) as ppool,
            tc.tile_pool(name="xkstream", bufs=4) as xkstream,
            tc.tile_pool(name="xvstream", bufs=2) as xvstream,
            tc.tile_pool(name="xqstream", bufs=2) as xqstream,
            tc.tile_pool(name="qtile", bufs=2) as qpool,
            tc.tile_pool(name="exp", bufs=6) as epool,
            tc.tile_pool(name="norm", bufs=4) as npool,
            tc.tile_pool(name="outsb", bufs=3) as opool,
            tc.tile_pool(name="s_psum", bufs=2, space="PSUM") as s_psum,
            tc.tile_pool(name="c_psum", bufs=2, space="PSUM") as c_psum,
            tc.tile_pool(name="p_psum", bufs=2, space="PSUM") as p_psum,
        ):
            kT_sb = ppool.tile([128, DC, S], bf16)
            v_sb = ppool.tile([128, KC, HL, DH + 1], bf16)
            ctxT_sb = ppool.tile([128, DC, S], bf16)
            wq_sb = wpool.tile([128, EC, G], bf16)
            wk_sb = wpool.tile([128, EC, G], bf16)
            wv_sb = wpool.tile([128, EC, G], bf16)
            wo_sb = wpool.tile([128, GC, E], bf16)
            bq_sb = wpool.tile([128, DC], f32)
            bk_sb = wpool.tile([128, DC], f32)
            mb_sb = wpool.tile([128, KC], f32)
            ones_row = wpool.tile([1, 64], bf16)

            # ones column for the denominator fusion: only column DH per head
            # needs presetting (the projection copies fill cols 0..DH-1)
            nc.gpsimd.memset(v_sb[:, :, :, DH : DH + 1], 1.0)
            nc.gpsimd.memset(ones_row[:], 1.0)

            def xstream(pool, src, lo, tag):
                t = pool.tile([128, EC, 512], bf16, tag=tag)
                nc.sync.dma_start(
                    t[:], src[:, lo : lo + 512].rearrange("(ec p) s -> p ec s", p=128)
                )
                return t

            # DMAs chunked and in need-order so wave-0 compute pipelines
            # behind the transfers: k inputs, v inputs, q inputs, wo last
            nc.sync.dma_start(bk_sb[:], bkd)
            nc.sync.dma_start(mb_sb[:], maskb)
            nc.sync.dma_start(bq_sb[:], bqd)
            wk_r = wkT.rearrange("(ec p) g -> p ec g", p=128)
            wv_r = wvT.rearrange("(ec p) g -> p ec g", p=128)
            xk0 = xkstream.tile([128, EC, 512], bf16, tag="xk")
            xv0 = xvstream.tile([128, EC, 512], bf16, tag="xv")
            xk_r = xkT[:, 0:512].rearrange("(ec p) s -> p ec s", p=128)
            xv_r = xvT[:, 0:512].rearrange("(ec p) s -> p ec s", p=128)
            for ec in range(EC):
                nc.sync.dma_start(wk_sb[:, ec, :], wk_r[:, ec, :])
                nc.sync.dma_start(xk0[:, ec, :], xk_r[:, ec, :])
            for ec in range(EC):
                nc.sync.dma_start(wv_sb[:, ec, :], wv_r[:, ec, :])
                nc.sync.dma_start(xv0[:, ec, :], xv_r[:, ec, :])
            nc.sync.dma_start(wq_sb[:], wqT.rearrange("(ec p) g -> p ec g", p=128))
            xk_ts = {0: xk0}

            # ---------- background work generators (yield ~per matmul) ----------
            def kproj_chunk(st, dc):
                ps = p_psum.tile([128, 512], f32, tag="proj", name=f"kp{st}_{dc}")
                for ec in range(EC):
                    nc.tensor.matmul(
                        ps[:],
                        lhsT=wk_sb[:, ec, dc * 128 : (dc + 1) * 128],
                        rhs=xk_ts[st][:, ec, :],
                        start=(ec == 0),
                        stop=(ec == EC - 1),
                    )
                    yield
                nc.vector.tensor_add(
                    out=kT_sb[:, dc, st * 512 : (st + 1) * 512],
                    in0=ps[:],
                    in1=bk_sb[:, dc : dc + 1].to_broadcast((128, 512)),
                )

            def vproj_chunk(xv_t, st, sci):
                sc = st * 4 + sci
                ps = p_psum.tile([128, 512], f32, tag="proj", name=f"vp{sc}")
                for ec in range(EC):
                    nc.tensor.matmul(
                        ps[:, :G],
                        lhsT=xv_t[:, ec, sci * 128 : (sci + 1) * 128],
                        rhs=wv_sb[:, ec, :],
                        start=(ec == 0),
                        stop=(ec == EC - 1),
                    )
                    yield
                nc.vector.tensor_copy(
                    out=v_sb[:, sc, :, 0:DH],
                    in_=ps[:, :G].rearrange("p (h d) -> p h d", h=HL),
                )

            qT_ts = {}
            xq_ts = {}

            def qproj_chunk(qt, dc):
                if qt not in qT_ts:
                    qT_ts[qt] = qpool.tile([128, DC, 512], bf16, tag="qT", name=f"qT{qt}")
                    xq_ts[qt] = xstream(xqstream, xqT, qt * 512, "xq")
                ps = p_psum.tile([128, 512], f32, tag="proj", name=f"qp{qt}_{dc}")
                for ec in range(EC):
                    nc.tensor.matmul(
                        ps[:],
                        lhsT=wq_sb[:, ec, dc * 128 : (dc + 1) * 128],
                        rhs=xq_ts[qt][:, ec, :],
                        start=(ec == 0),
                        stop=(ec == EC - 1),
                    )
                    yield
                nc.vector.tensor_add(
                    out=qT_ts[qt][:, dc, :],
                    in0=ps[:],
                    in1=bq_sb[:, dc : dc + 1].to_broadcast((128, 512)),
                )

            def outproj_chunk(st, ec, ps=None, scalar_copy=False):
                if ps is None:
                    ps = p_psum.tile([128, 512], f32, tag="proj", name=f"op{st}_{ec}")[:]
                for gc in range(GC):
                    nc.tensor.matmul(
                        ps,
                        lhsT=wo_sb[:, gc, ec * 128 : (ec + 1) * 128],
                        rhs=ctxT_sb[:, gc, st * 512 : (st + 1) * 512],
                        start=(gc == 0),
                        stop=(gc == GC - 1),
                    )
                    yield
                o_sb = opool.tile([128, 512], f32, tag="osb")
                if scalar_copy:
                    nc.scalar.copy(o_sb[:], ps)
                else:
                    nc.vector.tensor_copy(out=o_sb[:], in_=ps)
                nc.sync.dma_start(
                    out[ec * 128 : (ec + 1) * 128, st * 512 : (st + 1) * 512],
                    o_sb[:],
                )

            bg = []

            def drive(n=1):
                while n > 0 and bg:
                    try:
                        next(bg[0])
                        n -= 1
                    except StopIteration:
                        bg.pop(0)

            def drain_all():
                while bg:
                    drive(1)

            def force(g):
                while True:
                    try:
                        next(g)
                    except StopIteration:
                        break
                if g in bg:
                    bg.remove(g)

            # ---------- attention unit machinery (all state keyed per unit) ----
            ctx_ps = {}  # (qt, hp, hq) -> psum tile
            e_tiles = {}  # (qt, hp, kc) -> e tile
            pendq = {}  # (qt, hp) -> kcs whose ctx is not yet emitted (lag 2)

            def ctx_step(qt, hp, kc):
                """emit the M=65 ctx pair for kc (consumes its e tile)"""
                e = e_tiles.pop((qt, hp, kc))
                for hq in range(2):
                    if (qt, hp, hq) not in ctx_ps:
                        ctx_ps[(qt, hp, hq)] = c_psum.tile(
                            [128, 512], f32, tag="ctx", name=f"c{qt}_{hp}_{hq}"
                        )
                    nc.tensor.matmul(
                        ctx_ps[(qt, hp, hq)][0 : DH + 1, :],
                        lhsT=v_sb[:, kc, 2 * hp + hq, :],
                        rhs=e[:, 512 * hq : 512 * hq + 512],
                        start=(kc == 0),
                        stop=(kc == KC - 1),
                    )

            def att_kc(qt, hp, kc, bg_steps=2):
                """scores + exp for kc; ctx for kc-2."""
                qT_t = qT_ts[qt]
                k0 = kc * 128
                sp = s_psum.tile([128, 1024], f32, tag="sp")
                nc.tensor.matmul(
                    sp[:, 0:512],
                    lhsT=kT_sb[0:64, hp, k0 : k0 + 128],
                    rhs=qT_t[0:64, hp, :],
                    start=True,
                    stop=True,
                )
                nc.tensor.matmul(
                    sp[:, 512:1024],
                    lhsT=kT_sb[64:128, hp, k0 : k0 + 128],
                    rhs=qT_t[64:128, hp, :],
                    start=True,
                    stop=True,
                )
                e = epool.tile([128, 1024], bf16, tag="exp")
                nc.scalar.activation(
                    e[:], sp[:], AF.Exp,
                    bias=mb_sb[:, kc : kc + 1], scale=float(SCALE),
                )
                e_tiles[(qt, hp, kc)] = e
                q = pendq.setdefault((qt, hp), [])
                q.append(kc)
                if len(q) > 2:
                    ctx_step(qt, hp, q.pop(0))
                drive(bg_steps)

            def att_close_gen(qt, hp):
                """two-phase close: (A) final ctx, evacuation, reciprocals;
                (B) partition-broadcast of 1/den via K=1 matmuls + normalize."""
                q = pendq.pop((qt, hp))
                while q:
                    ctx_step(qt, hp, q.pop(0))
                qs = slice(qt * 512, qt * 512 + 512)
                recs = []
                for hq in range(2):
                    cp = ctx_ps.pop((qt, hp, hq))
                    # evacuate ctx (frees the bank) then normalize in SBUF
                    nc.vector.tensor_copy(
                        out=ctxT_sb[64 * hq : 64 * hq + 64, hp, qs], in_=cp[0:DH, :]
                    )
                    den = npool.tile([1, 512], f32, tag="den")
                    nc.vector.tensor_copy(out=den[:], in_=cp[DH : DH + 1, :])
                    rec = npool.tile([1, 512], f32, tag="rec")
                    nc.vector.reciprocal_approx_fast(rec[:], den[:])
                    recb = npool.tile([1, 512], bf16, tag="recb")
                    nc.vector.tensor_copy(out=recb[:], in_=rec[:])
                    recs.append(recb)
                yield
                # rb[p, q] = 1/den[head(p), q] via ones[1,64].T @ rec[1,512]
                rb_ps = p_psum.tile([128, 512], f32, tag="proj", name=f"rb{qt}_{hp}")
                for hq in range(2):
                    nc.tensor.matmul(
                        rb_ps[64 * hq : 64 * hq + 64, :],
                        lhsT=ones_row[0:1, :],
                        rhs=recs[hq][:],
                        start=True,
                        stop=True,
                    )
                nc.vector.tensor_mul(
                    out=ctxT_sb[:, hp, qs],
                    in0=ctxT_sb[:, hp, qs],
                    in1=rb_ps[:],
                )

            # ---------- schedule ----------
            # prologue: wave 0 of k/v projections, ec-outer over 4 psum
            # accumulators so the matmuls pipeline behind the chunked DMAs
            kps = [
                p_psum.tile([128, 512], f32, tag="proj", name=f"k0_{d}")
                for d in range(2)
            ] + [
                c_psum.tile([128, 512], f32, tag="ctx", name=f"k0_{d + 2}")
                for d in range(2)
            ]
            for ec in range(EC):
                for dc in range(DC):
                    nc.tensor.matmul(
                        kps[dc][:],
                        lhsT=wk_sb[:, ec, dc * 128 : (dc + 1) * 128],
                        rhs=xk0[:, ec, :],
                        start=(ec == 0),
                        stop=(ec == EC - 1),
                    )
            for dc in range(DC):
                nc.vector.tensor_add(
                    out=kT_sb[:, dc, 0:512],
                    in0=kps[dc][:],
                    in1=bk_sb[:, dc : dc + 1].to_broadcast((128, 512)),
                )
            vps = [
                p_psum.tile([128, 512], f32, tag="proj", name=f"v0_{d}")
                for d in range(2)
            ] + [
                c_psum.tile([128, 512], f32, tag="ctx", name=f"v0_{d + 2}")
                for d in range(2)
            ]
            for ec in range(EC):
                for sci in range(4):
                    nc.tensor.matmul(
                        vps[sci][:],
                        lhsT=xv0[:, ec, sci * 128 : (sci + 1) * 128],
                        rhs=wv_sb[:, ec, :],
                        start=(ec == 0),
                        stop=(ec == EC - 1),
                    )
            for sci in range(4):
                nc.vector.tensor_copy(
                    out=v_sb[:, sci, :, 0:DH],
                    in_=vps[sci][:].rearrange("p (h d) -> p h d", h=HL),
                )
            qp_gens = {(0, dc): qproj_chunk(0, dc) for dc in range(DC)}
            force(qp_gens.pop((0, 0)))
            nc.sync.dma_start(wo_sb[:], woT.rearrange("(gc p) e -> p gc e", p=128))
            # waves 1..3: vproj + per-dc kproj, forced as late as possible
            wave_vp = {}
            wave_kp = {}
            for st in range(1, QT):
                xk_ts[st] = xstream(xkstream, xkT, st * 512, "xk")
                xv_s = xstream(xvstream, xvT, st * 512, "xv")
                wave_kp[st] = {dc: kproj_chunk(st, dc) for dc in range(DC)}
                wave_vp[st] = [vproj_chunk(xv_s, st, sci) for sci in range(4)]
                bg.append(wave_kp[st][0])
                bg.extend(wave_vp[st])
            for st in range(1, QT):
                for dc in range(1, DC):
                    bg.append(wave_kp[st][dc])
            bg.append(qp_gens[(0, 1)])
            bg.append(qp_gens[(0, 2)])
            bg.append(qp_gens[(0, 3)])

            def ensure_wave(st, hp):
                """emit everything attention (qt0, hp) needs for k-chunks of st"""
                if st == 0:
                    return
                for g in wave_vp[st]:
                    force(g)
                for dc in range(hp + 1):
                    force(wave_kp[st][dc])

            def unit_begin(qt, hp):
                g = qp_gens.pop((qt, hp), None)
                if g is not None:
                    force(g)

            # closes deferred into the next unit's first k-chunks, one phase
            # per pop so the broadcast matmuls never stall on the reciprocals
            closes = []

            def pop_close():
                if closes:
                    try:
                        next(closes[0])
                    except StopIteration:
                        closes.pop(0)

            for qt in range(QT):
                for hp in range(DC):
                    unit_begin(qt, hp)
                    if qt >= 1 and hp == 1:
                        # qt-1 rows of ctxT are final: output projection
                        for hh in range(DC):
                            bg.append(outproj_chunk(qt - 1, 2 * hh))
                            bg.append(outproj_chunk(qt - 1, 2 * hh + 1))
                    for kc in range(KC):
                        if qt == 0 and kc % 4 == 0:
                            ensure_wave(kc // 4, hp)
                        att_kc(qt, hp, kc, bg_steps=2)
                        if kc in (0, 3):
                            pop_close()
                    closes.append(att_close_gen(qt, hp))
                    if qt < QT - 1:
                        qp_gens[(qt + 1, hp)] = qproj_chunk(qt + 1, hp)
                        bg.append(qp_gens[(qt + 1, hp)])
            while closes:
                pop_close()

            # tail: final output projection over rotating psum banks
            drain_all()
            s1 = s_psum.tile([128, 1024], f32, tag="sp", name="tail1")
            s2 = s_psum.tile([128, 1024], f32, tag="sp", name="tail2")
            tail_ps = [
                p_psum.tile([128, 512], f32, tag="proj", name="tp0")[:],
                p_psum.tile([128, 512], f32, tag="proj", name="tp1")[:],
                c_psum.tile([128, 512], f32, tag="ctx", name="tc0")[:],
                c_psum.tile([128, 512], f32, tag="ctx", name="tc1")[:],
                s1[:, 0:512],
                s1[:, 512:1024],
                s2[:, 0:512],
                s2[:, 512:1024],
            ]
            tail_gens = [
                outproj_chunk(QT - 1, ec, ps=tail_ps[ec], scalar_copy=(ec % 2 == 1))
                for ec in range(EC)
            ]
            alive = list(tail_gens)
            while alive:
                for g in list(alive):
                    try:
                        next(g)
                    except StopIteration:
                        alive.remove(g)

    nc.compile()
    return nc


def _prep_core_inputs(query, key, value, mask, Wq, bq, Wk, bk, Wv, Wo):
    """Per-core input maps: core c -> batch c//2, head-group c%2."""
    import ml_dtypes

    f = ml_dtypes.bfloat16
    maps = []
    for c in range(8):
        b, g = c // 2, c % 2
        lo = g * G
        mrow = mask[b, 0].astype(np.float64)
        maskb = np.where(mrow == 0, MASK_NEG, 0.0).reshape(KC, 128).T
        maps.append(
            {
                "xqT": np.ascontiguousarray(query[b].T).astype(f, copy=False),
                "xkT": np.ascontiguousarray(key[b].T).astype(f, copy=False),
                "xvT": np.ascontiguousarray(value[b].T).astype(f, copy=False),
                "wqT": np.ascontiguousarray(Wq[lo : lo + G].T).astype(f, copy=False),
                "wkT": np.ascontiguousarray(Wk[lo : lo + G].T).astype(f, copy=False),
                "wvT": np.ascontiguousarray(Wv[lo : lo + G].T).astype(f, copy=False),
                "woT": np.ascontiguousarray(Wo[:, lo : lo + G].T).astype(f, copy=False),
                "bqd": np.ascontiguousarray(bq[lo : lo + G].reshape(DC, 128).T).astype(np.float32),
                "bkd": np.ascontiguousarray(bk[lo : lo + G].reshape(DC, 128).T).astype(np.float32),
                "maskb": np.ascontiguousarray(maskb).astype(np.float32),
            }
        )
    return maps


def kernel(query, key, value, mask, Wq, bq, Wk, bk, Wv, bv, Wo, bo, _results=None):
    global _NC
    query = np.asarray(query, dtype=np.float32)
    key = np.asarray(key, dtype=np.float32)
    value = np.asarray(value, dtype=np.float32)
    mask = np.asarray(mask)
    Wq, bq = np.asarray(Wq, np.float32), np.asarray(bq, np.float32)
    Wk, bk = np.asarray(Wk, np.float32), np.asarray(bk, np.float32)
    Wv, bv = np.asarray(Wv, np.float32), np.asarray(bv, np.float32)
    Wo, bo = np.asarray(Wo, np.float32), np.asarray(bo, np.float32)

    if _NC is None:
        _NC = _build_program()
    in_maps = _prep_core_inputs(query, key, value, mask, Wq, bq, Wk, bk, Wv, Wo)
    res = run_bass_kernel_spmd(_NC, in_maps, core_ids=list(range(8)))
    if _results is not None:
        _results.append(res)

    # host epilogue: sum the two head-group partials; bv commutes with softmax
    # (rows sum to 1) so its contribution is Wo @ bv, plus the output bias bo.
    extra = (Wo.astype(np.float64) @ bv.astype(np.float64) + bo.astype(np.float64)).astype(
        np.float32
    )
    out = np.empty((B, S, E), dtype=np.float32)
    for b in range(B):
        out[b] = (
            res.results[2 * b]["out"] + res.results[2 * b + 1]["out"]
        ).T + extra
    return out


# revision 29
# speedup vs baseline: 1.0199x; 1.0199x over previous
"""Multi-head attention (B=4, S=2048, D=1024, H=16) on 8 Trainium2 NeuronCores.

Sharding: core c handles batch c//2 and head-group c%2 (8 heads = 512 dims of
the per-head concat). Each core computes its q/k/v projections (tensor
parallel over heads), attention for its 8 heads, and a partial output
projection over its 512 concat dims; the host sums the two partials per batch.

v3 dataflow (all matmuls bf16, f32 psum accumulate):
  - scores computed transposed S^T[k, q] so the softmax mask/bias is a
    per-partition ACT bias and exp(scale*s + bias) is one ACT op; the two
    K=64 head-halves run row-packed (concurrent row groups).
  - ctx^T = [V | 1]^T @ e accumulated over k-chunks, M=65: psum row 64 is
    the softmax denominator (ones column rides the contraction for free).
  - ctx matmuls lag the exp by TWO k-chunks so the PE stream never waits
    on the ACT semaphore (scores for kc are emitted ahead of ctx(kc-2)).
  - k/v/q projections and the transposed output projection run as
    background generators inside the attention loop; unit closes
    (normalization) are deferred into the next unit's first k-chunks so
    the ACT queue never drains at unit boundaries.
  - exp instructions are the serial resource: 256 x [128,1024] on the ACT
    engine (~1.1us each) bound the kernel; everything else hides under it.

PSUM banks: scores 2x[128,1024]=4, ctx pair (M=65) 2, proj 2.

Host epilogue: out[b] = partial[2b] + partial[2b+1] + (Wo @ bv + bo); the
value bias commutes with softmax (rows sum to 1) so it is exact. Key/query
biases applied on-device.
"""

import sys

sys.path.insert(0, "/opt/trn_rl_repo")

import numpy as np

import concourse.bacc as bacc
import concourse.mybir as mybir
import concourse.tile as tile
from concourse.bass_utils import run_bass_kernel_spmd

f32 = mybir.dt.float32
bf16 = mybir.dt.bfloat16
AF = mybir.ActivationFunctionType

B, S, E, H = 4, 2048, 1024, 16
DH = E // H  # 64
G = E // 2  # 512 dims per core (8 heads)
HL = H // 2  # heads per core
EC = E // 128  # 8 e-chunks (projection contraction)
DC = G // 128  # 4 head-pairs per core
QT = S // 512  # 4 q-tiles
KC = S // 128  # 16 k-chunks
GC = G // 128  # 4 chunks of the local concat dim (out-proj contraction)
SCALE = 1.0 / np.sqrt(np.float64(E))
MASK_NEG = -88.0  # exp(-88 + |s|max) == 0 in fp32 for masked keys

_NC = None


def _build_program():
    nc = bacc.Bacc("TRN2", target_bir_lowering=False, debug=False, num_devices=8)

    # streams pre-tiled host-side: [s-tile, E, 512], contiguous per tile
    xqT = nc.dram_tensor("xqT", [QT, E, 512], bf16, kind="ExternalInput").ap()
    xkT = nc.dram_tensor("xkT", [QT, E, 512], bf16, kind="ExternalInput").ap()
    xvT = nc.dram_tensor("xvT", [QT, E, 512], bf16, kind="ExternalInput").ap()
    wqT = nc.dram_tensor("wqT", [E, G], bf16, kind="ExternalInput").ap()
    wkT = nc.dram_tensor("wkT", [E, G], bf16, kind="ExternalInput").ap()
    wvT = nc.dram_tensor("wvT", [E, G], bf16, kind="ExternalInput").ap()
    woT = nc.dram_tensor("woT", [G, E], bf16, kind="ExternalInput").ap()
    bqd = nc.dram_tensor("bqd", [128, DC], f32, kind="ExternalInput").ap()
    bkd = nc.dram_tensor("bkd", [128, DC], f32, kind="ExternalInput").ap()
    maskb = nc.dram_tensor("maskb", [128, KC], f32, kind="ExternalInput").ap()
    out = nc.dram_tensor("out", [E, S], f32, kind="ExternalOutput").ap()  # transposed

    with tile.TileContext(nc) as tc:
        with (
            tc.tile_pool(name="weights", bufs=1) as wpool,
            tc.tile_pool(name="persist", bufs=1) as ppool,
            tc.tile_pool(name="xkstream", bufs=4) as xkstream,
            tc.tile_pool(name="xvstream", bufs=2) as xvstream,
            tc.tile_pool(name="xqstream", bufs=2) as xqstream,
            tc.tile_pool(name="qtile", bufs=2) as qpool,
            tc.tile_pool(name="exp", bufs=6) as epool,
            tc.tile_pool(name="norm", bufs=4) as npool,
            tc.tile_pool(name="outsb", bufs=3) as opool,
            tc.tile_pool(name="s_psum", bufs=2, space="PSUM") as s_psum,
            tc.tile_pool(name="c_psum", bufs=2, space="PSUM") as c_psum,
            tc.tile_pool(name="p_psum", bufs=2, space="PSUM") as p_psum,
        ):
            kT_sb = ppool.tile([128, DC, S], bf16)
            v_sb = ppool.tile([128, KC, HL, DH + 1], bf16)
            ctxT_sb = ppool.tile([128, DC, S], bf16)
            wq_sb = wpool.tile([128, EC, G], bf16)
            wk_sb = wpool.tile([128, EC, G], bf16)
            wv_sb = wpool.tile([128, EC, G], bf16)
            wo_sb = wpool.tile([128, GC, E], bf16)
            bq_sb = wpool.tile([128, DC], f32)
            bk_sb = wpool.tile([128, DC], f32)
            mb_sb = wpool.tile([128, KC], f32)
            ones_row = wpool.tile([1, 64], bf16)

            # ones column for the denominator fusion: only column DH per head
            # needs presetting (the projection copies fill cols 0..DH-1)
            nc.gpsimd.memset(v_sb[:, :, :, DH : DH + 1], 1.0)
            nc.gpsimd.memset(ones_row[:], 1.0)

            def xstream(pool, src, st, tag):
                t = pool.tile([128, EC, 512], bf16, tag=tag)
                nc.sync.dma_start(
                    t[:], src[st].rearrange("(ec p) s -> p ec s", p=128)
                )
                return t

            # whole-tile DMAs in need-order: k inputs, v inputs, q inputs
            nc.sync.dma_start(wk_sb[:], wkT.rearrange("(ec p) g -> p ec g", p=128))
            xk0 = xstream(xkstream, xkT, 0, "xk")
            nc.sync.dma_start(bk_sb[:], bkd)
            nc.sync.dma_start(mb_sb[:], maskb)
            nc.sync.dma_start(bq_sb[:], bqd)
            nc.sync.dma_start(wv_sb[:], wvT.rearrange("(ec p) g -> p ec g", p=128))
            xv0 = xstream(xvstream, xvT, 0, "xv")
            nc.sync.dma_start(wq_sb[:], wqT.rearrange("(ec p) g -> p ec g", p=128))
            xk_ts = {0: xk0}

            # ---------- background work generators (yield ~per matmul) ----------
            def kproj_chunk(st, dc):
                ps = p_psum.tile([128, 512], f32, tag="proj", name=f"kp{st}_{dc}")
                for ec in range(EC):
                    nc.tensor.matmul(
                        ps[:],
                        lhsT=wk_sb[:, ec, dc * 128 : (dc + 1) * 128],
                        rhs=xk_ts[st][:, ec, :],
                        start=(ec == 0),
                        stop=(ec == EC - 1),
                    )
                    yield
                nc.vector.tensor_add(
                    out=kT_sb[:, dc, st * 512 : (st + 1) * 512],
                    in0=ps[:],
                    in1=bk_sb[:, dc : dc + 1].to_broadcast((128, 512)),
                )

            def vproj_chunk(xv_t, st, sci):
                sc = st * 4 + sci
                ps = p_psum.tile([128, 512], f32, tag="proj", name=f"vp{sc}")
                for ec in range(EC):
                    nc.tensor.matmul(
                        ps[:, :G],
                        lhsT=xv_t[:, ec, sci * 128 : (sci + 1) * 128],
                        rhs=wv_sb[:, ec, :],
                        start=(ec == 0),
                        stop=(ec == EC - 1),
                    )
                    yield
                nc.vector.tensor_copy(
                    out=v_sb[:, sc, :, 0:DH],
                    in_=ps[:, :G].rearrange("p (h d) -> p h d", h=HL),
                )

            qT_ts = {}
            xq_ts = {}

            def qproj_chunk(qt, dc):
                if qt not in qT_ts:
                    qT_ts[qt] = qpool.tile([128, DC, 512], bf16, tag="qT", name=f"qT{qt}")
                    xq_ts[qt] = xstream(xqstream, xqT, qt, "xq")
                ps = p_psum.tile([128, 512], f32, tag="proj", name=f"qp{qt}_{dc}")
                for ec in range(EC):
                    nc.tensor.matmul(
                        ps[:],
                        lhsT=wq_sb[:, ec, dc * 128 : (dc + 1) * 128],
                        rhs=xq_ts[qt][:, ec, :],
                        start=(ec == 0),
                        stop=(ec == EC - 1),
                    )
                    yield
                nc.vector.tensor_add(
                    out=qT_ts[qt][:, dc, :],
                    in0=ps[:],
                    in1=bq_sb[:, dc : dc + 1].to_broadcast((128, 512)),
                )

            def outproj_chunk(st, ec, ps=None, scalar_copy=False):
                if ps is None:
                    ps = p_psum.tile([128, 512], f32, tag="proj", name=f"op{st}_{ec}")[:]
                for gc in range(GC):
                    nc.tensor.matmul(
                        ps,
                        lhsT=wo_sb[:, gc, ec * 128 : (ec + 1) * 128],
                        rhs=ctxT_sb[:, gc, st * 512 : (st + 1) * 512],
                        start=(gc == 0),
                        stop=(gc == GC - 1),
                    )
                    yield
                o_sb = opool.tile([128, 512], f32, tag="osb")
                if scalar_copy:
                    nc.scalar.copy(o_sb[:], ps)
                else:
                    nc.vector.tensor_copy(out=o_sb[:], in_=ps)
                nc.sync.dma_start(
                    out[ec * 128 : (ec + 1) * 128, st * 512 : (st + 1) * 512],
                    o_sb[:],
                )

            bg = []

            def drive(n=1):
                while n > 0 and bg:
                    try:
                        next(bg[0])
                        n -= 1
                    except StopIteration:
                        bg.pop(0)

            def drain_all():
                while bg:
                    drive(1)

            def force(g):
                while True:
                    try:
                        next(g)
                    except StopIteration:
                        break
                if g in bg:
                    bg.remove(g)

            # ---------- attention unit machinery (all state keyed per unit) ----
            ctx_ps = {}  # (qt, hp, hq) -> psum tile
            e_tiles = {}  # (qt, hp, kc) -> e tile
            pendq = {}  # (qt, hp) -> kcs whose ctx is not yet emitted (lag 2)

            def ctx_step(qt, hp, kc):
                """emit the M=65 ctx pair for kc (consumes its e tile)"""
                e = e_tiles.pop((qt, hp, kc))
                for hq in range(2):
                    if (qt, hp, hq) not in ctx_ps:
                        ctx_ps[(qt, hp, hq)] = c_psum.tile(
                            [128, 512], f32, tag="ctx", name=f"c{qt}_{hp}_{hq}"
                        )
                    nc.tensor.matmul(
                        ctx_ps[(qt, hp, hq)][0 : DH + 1, :],
                        lhsT=v_sb[:, kc, 2 * hp + hq, :],
                        rhs=e[:, 512 * hq : 512 * hq + 512],
                        start=(kc == 0),
                        stop=(kc == KC - 1),
                    )

            def att_kc(qt, hp, kc, bg_steps=2):
                """scores + exp for kc; ctx for kc-2."""
                qT_t = qT_ts[qt]
                k0 = kc * 128
                sp = s_psum.tile([128, 1024], f32, tag="sp")
                nc.tensor.matmul(
                    sp[:, 0:512],
                    lhsT=kT_sb[0:64, hp, k0 : k0 + 128],
                    rhs=qT_t[0:64, hp, :],
                    start=True,
                    stop=True,
                )
                nc.tensor.matmul(
                    sp[:, 512:1024],
                    lhsT=kT_sb[64:128, hp, k0 : k0 + 128],
                    rhs=qT_t[64:128, hp, :],
                    start=True,
                    stop=True,
                )
                e = epool.tile([128, 1024], bf16, tag="exp")
                nc.scalar.activation(
                    e[:], sp[:], AF.Exp,
                    bias=mb_sb[:, kc : kc + 1], scale=float(SCALE),
                )
                e_tiles[(qt, hp, kc)] = e
                q = pendq.setdefault((qt, hp), [])
                q.append(kc)
                if len(q) > 2:
                    ctx_step(qt, hp, q.pop(0))
                drive(bg_steps)

            def att_close_gen(qt, hp):
                """two-phase close: (A) final ctx, evacuation, reciprocals;
                (B) partition-broadcast of 1/den via K=1 matmuls + normalize."""
                q = pendq.pop((qt, hp))
                while q:
                    ctx_step(qt, hp, q.pop(0))
                qs = slice(qt * 512, qt * 512 + 512)
                recs = []
                for hq in range(2):
                    cp = ctx_ps.pop((qt, hp, hq))
                    # evacuate ctx (frees the bank) then normalize in SBUF
                    nc.vector.tensor_copy(
                        out=ctxT_sb[64 * hq : 64 * hq + 64, hp, qs], in_=cp[0:DH, :]
                    )
                    den = npool.tile([1, 512], f32, tag="den")
                    nc.vector.tensor_copy(out=den[:], in_=cp[DH : DH + 1, :])
                    rec = npool.tile([1, 512], f32, tag="rec")
                    nc.vector.reciprocal_approx_fast(rec[:], den[:])
                    recb = npool.tile([1, 512], bf16, tag="recb")
                    nc.vector.tensor_copy(out=recb[:], in_=rec[:])
                    recs.append(recb)
                yield
                # rb[p, q] = 1/den[head(p), q] via ones[1,64].T @ rec[1,512]
                rb_ps = p_psum.tile([128, 512], f32, tag="proj", name=f"rb{qt}_{hp}")
                for hq in range(2):
                    nc.tensor.matmul(
                        rb_ps[64 * hq : 64 * hq + 64, :],
                        lhsT=ones_row[0:1, :],
                        rhs=recs[hq][:],
                        start=True,
                        stop=True,
                    )
                nc.vector.tensor_mul(
                    out=ctxT_sb[:, hp, qs],
                    in0=ctxT_sb[:, hp, qs],
                    in1=rb_ps[:],
                )

            # ---------- schedule ----------
            # prologue: wave 0 of k/v projections, ec-outer over 4 psum
            # accumulators so the matmuls pipeline behind the chunked DMAs
            kps = [
                p_psum.tile([128, 512], f32, tag="proj", name=f"k0_{d}")
                for d in range(2)
            ] + [
                c_psum.tile([128, 512], f32, tag="ctx", name=f"k0_{d + 2}")
                for d in range(2)
            ]
            for ec in range(EC):
                for dc in range(DC):
                    nc.tensor.matmul(
                        kps[dc][:],
                        lhsT=wk_sb[:, ec, dc * 128 : (dc + 1) * 128],
                        rhs=xk0[:, ec, :],
                        start=(ec == 0),
                        stop=(ec == EC - 1),
                    )
            for dc in range(DC):
                nc.vector.tensor_add(
                    out=kT_sb[:, dc, 0:512],
                    in0=kps[dc][:],
                    in1=bk_sb[:, dc : dc + 1].to_broadcast((128, 512)),
                )
            vps = [
                p_psum.tile([128, 512], f32, tag="proj", name=f"v0_{d}")
                for d in range(2)
            ] + [
                c_psum.tile([128, 512], f32, tag="ctx", name=f"v0_{d + 2}")
                for d in range(2)
            ]
            for ec in range(EC):
                for sci in range(4):
                    nc.tensor.matmul(
                        vps[sci][:],
                        lhsT=xv0[:, ec, sci * 128 : (sci + 1) * 128],
                        rhs=wv_sb[:, ec, :],
                        start=(ec == 0),
                        stop=(ec == EC - 1),
                    )
            for sci in range(4):
                nc.vector.tensor_copy(
                    out=v_sb[:, sci, :, 0:DH],
                    in_=vps[sci][:].rearrange("p (h d) -> p h d", h=HL),
                )
            qp_gens = {(0, dc): qproj_chunk(0, dc) for dc in range(DC)}
            force(qp_gens.pop((0, 0)))
            nc.sync.dma_start(wo_sb[:], woT.rearrange("(gc p) e -> p gc e", p=128))
            # waves 1..3: vproj + per-dc kproj, forced as late as possible
            wave_vp = {}
            wave_kp = {}
            for st in range(1, QT):
                xk_ts[st] = xstream(xkstream, xkT, st, "xk")
                xv_s = xstream(xvstream, xvT, st, "xv")
                wave_kp[st] = {dc: kproj_chunk(st, dc) for dc in range(DC)}
                wave_vp[st] = [vproj_chunk(xv_s, st, sci) for sci in range(4)]
                bg.append(wave_kp[st][0])
                bg.extend(wave_vp[st])
            for st in range(1, QT):
                for dc in range(1, DC):
                    bg.append(wave_kp[st][dc])
            bg.append(qp_gens[(0, 1)])
            bg.append(qp_gens[(0, 2)])
            bg.append(qp_gens[(0, 3)])

            def ensure_wave(st, hp):
                """emit everything attention (qt0, hp) needs for k-chunks of st"""
                if st == 0:
                    return
                for g in wave_vp[st]:
                    force(g)
                for dc in range(hp + 1):
                    force(wave_kp[st][dc])

            def unit_begin(qt, hp):
                g = qp_gens.pop((qt, hp), None)
                if g is not None:
                    force(g)

            # closes deferred into the next unit's first k-chunks, one phase
            # per pop so the broadcast matmuls never stall on the reciprocals
            closes = []

            def pop_close():
                if closes:
                    try:
                        next(closes[0])
                    except StopIteration:
                        closes.pop(0)

            for qt in range(QT):
                for hp in range(DC):
                    unit_begin(qt, hp)
                    if qt >= 1 and hp == 1:
                        # qt-1 rows of ctxT are final: output projection
                        for hh in range(DC):
                            bg.append(outproj_chunk(qt - 1, 2 * hh))
                            bg.append(outproj_chunk(qt - 1, 2 * hh + 1))
                    for kc in range(KC):
                        if qt == 0 and kc % 4 == 0:
                            ensure_wave(kc // 4, hp)
                        att_kc(qt, hp, kc, bg_steps=2)
                        if kc in (0, 3):
                            pop_close()
                    closes.append(att_close_gen(qt, hp))
                    if qt < QT - 1:
                        qp_gens[(qt + 1, hp)] = qproj_chunk(qt + 1, hp)
                        bg.append(qp_gens[(qt + 1, hp)])
            while closes:
                pop_close()

            # tail: final output projection over rotating psum banks
            drain_all()
            s1 = s_psum.tile([128, 1024], f32, tag="sp", name="tail1")
            s2 = s_psum.tile([128, 1024], f32, tag="sp", name="tail2")
            tail_ps = [
                p_psum.tile([128, 512], f32, tag="proj", name="tp0")[:],
                p_psum.tile([128, 512], f32, tag="proj", name="tp1")[:],
                c_psum.tile([128, 512], f32, tag="ctx", name="tc0")[:],
                c_psum.tile([128, 512], f32, tag="ctx", name="tc1")[:],
                s1[:, 0:512],
                s1[:, 512:1024],
                s2[:, 0:512],
                s2[:, 512:1024],
            ]
            tail_gens = [
                outproj_chunk(QT - 1, ec, ps=tail_ps[ec], scalar_copy=(ec % 2 == 1))
                for ec in range(EC)
            ]
            alive = list(tail_gens)
            while alive:
                for g in list(alive):
                    try:
                        next(g)
                    except StopIteration:
                        alive.remove(g)

    nc.compile()
    return nc


def _prep_core_inputs(query, key, value, mask, Wq, bq, Wk, bk, Wv, Wo):
    """Per-core input maps: core c -> batch c//2, head-group c%2."""
    import ml_dtypes

    f = ml_dtypes.bfloat16
    maps = []
    for c in range(8):
        b, g = c // 2, c % 2
        lo = g * G
        mrow = mask[b, 0].astype(np.float64)
        maskb = np.where(mrow == 0, MASK_NEG, 0.0).reshape(KC, 128).T
        maps.append(
            {
                "xqT": np.ascontiguousarray(
                    query[b].T.reshape(E, QT, 512).transpose(1, 0, 2)
                ).astype(f, copy=False),
                "xkT": np.ascontiguousarray(
                    key[b].T.reshape(E, QT, 512).transpose(1, 0, 2)
                ).astype(f, copy=False),
                "xvT": np.ascontiguousarray(
                    value[b].T.reshape(E, QT, 512).transpose(1, 0, 2)
                ).astype(f, copy=False),
                "wqT": np.ascontiguousarray(Wq[lo : lo + G].T).astype(f, copy=False),
                "wkT": np.ascontiguousarray(Wk[lo : lo + G].T).astype(f, copy=False),
                "wvT": np.ascontiguousarray(Wv[lo : lo + G].T).astype(f, copy=False),
                "woT": np.ascontiguousarray(Wo[:, lo : lo + G].T).astype(f, copy=False),
                "bqd": np.ascontiguousarray(bq[lo : lo + G].reshape(DC, 128).T).astype(np.float32),
                "bkd": np.ascontiguousarray(bk[lo : lo + G].reshape(DC, 128).T).astype(np.float32),
                "maskb": np.ascontiguousarray(maskb).astype(np.float32),
            }
        )
    return maps


def kernel(query, key, value, mask, Wq, bq, Wk, bk, Wv, bv, Wo, bo, _results=None):
    global _NC
    query = np.asarray(query, dtype=np.float32)
    key = np.asarray(key, dtype=np.float32)
    value = np.asarray(value, dtype=np.float32)
    mask = np.asarray(mask)
    Wq, bq = np.asarray(Wq, np.float32), np.asarray(bq, np.float32)
    Wk, bk = np.asarray(Wk, np.float32), np.asarray(bk, np.float32)
    Wv, bv = np.asarray(Wv, np.float32), np.asarray(bv, np.float32)
    Wo, bo = np.asarray(Wo, np.float32), np.asarray(bo, np.float32)

    if _NC is None:
        _NC = _build_program()
    in_maps = _prep_core_inputs(query, key, value, mask, Wq, bq, Wk, bk, Wv, Wo)
    res = run_bass_kernel_spmd(_NC, in_maps, core_ids=list(range(8)))
    if _results is not None:
        _results.append(res)

    # host epilogue: sum the two head-group partials; bv commutes with softmax
    # (rows sum to 1) so its contribution is Wo @ bv, plus the output bias bo.
    extra = (Wo.astype(np.float64) @ bv.astype(np.float64) + bo.astype(np.float64)).astype(
        np.float32
    )
    out = np.empty((B, S, E), dtype=np.float32)
    for b in range(B):
        out[b] = (
            res.results[2 * b]["out"] + res.results[2 * b + 1]["out"]
        ).T + extra
    return out
